# revision 33
# baseline (speedup 1.0000x reference)
"""HardTripletLoss Trainium2 kernel.

Reference computation (B=256, C=1000, D=300):
  relations[b,c] = ||emb[b*C+c] - att[b*C+c] + 1e-6||_2          [B, C]
  hardest_positive[c] = max_b relations[b,c] * onehot(labels)[b,c]
  mx[c]              = max_b relations[b,c]
  hardest_negative[c] = min_b (relations[b,c] + mx[c]*onehot[b,c])
  loss = sum(relu(hp - hn + 1)) / (count(relu(...) > 1e-16) + 1e-16)

Sharding: data-parallel over B across 8 cores; core m owns the contiguous
32000-row slice [m*32000, (m+1)*32000) of the (B*C, D) tensors. The device
does the heavy part only: per-row squared distances rel_sq[row] =
sum_d (emb-att+eps)^2, shipped back as [128, 250] f32 (128 KB/core,
+0.3% traffic). The host reshapes to [B, C], takes sqrt, and runs the
reference's tiny [B,C] max/min/mask logic in numpy (~ms).

Performance design (HW exec ~= DMA roofline; measured cap ~200 GB/s/core):
- fp16 inputs (host pre-cast): halves device bytes to 38.4 MB/core.
  Final loss err ~3e-5 vs 2e-2 tolerance.
- All bulk loads on SWDGE (gpsimd): packets round-robin over all 16 DMA
  engines (22.5 GB/s each). HWDGE queues are pinned to engines 64-68
  only and bottleneck at ~110 GB/s.
- Rows-contiguous layout: partition p holds rows [p*250, (p+1)*250), so
  a 25-row chunk DMA is [128, 15 KB] -- 128 descriptors, 15 KB packets.
  Fewer descriptors per DMA matters because SWDGE issue lock-steps on a
  small completion-sem ring (~8 DMAs in flight): with 250-desc pair DMAs
  the issue stream stalled 10-14 us at a time; 20 chunk DMAs of 1.92 MB
  keep ~15 MB queued ahead of the engines.
- Per chunk: in-place DVE subtract, one ACT Square (bias=eps), two f16
  fold-adds (2 elem/cycle) + one f32 3D row-sum reduce on DVE. All
  compute engines run far below the DMA window.
"""

import numpy as np

B, C, D = 256, 1000, 300
M = 8              # cores
BL = B // M        # 32 local anchors per core
ROWS = BL * C      # 32000 rows per core
P = 128            # partitions; partition p holds rows [p*RPP, (p+1)*RPP)
RPP = ROWS // P    # 250 rows per partition
RPC = 25           # rows per chunk (per partition)
NCH = RPP // RPC   # 10 chunks
EPS_PD = 1e-6
MARGIN = 1.0

_STATE = {}


def _build():
    import concourse.tile as tile
    from concourse import bacc, mybir

    nc = bacc.Bacc("TRN2", target_bir_lowering=False, debug=False,
                   num_devices=M, num_swdge_queues=4)
    dt = mybir.dt.float32
    dt16 = mybir.dt.float16
    emb = nc.dram_tensor("emb", [ROWS, D], dt16, kind="ExternalInput").ap()
    att = nc.dram_tensor("att", [ROWS, D], dt16, kind="ExternalInput").ap()
    out = nc.dram_tensor("out", [P, RPP], dt, kind="ExternalOutput").ap()

    # row = p*250 + w  ->  per-partition row view [p, w, d]; chunks are
    # uneven: small ones first (compute starts sooner after launch) and
    # last (short serial drain chain after the final DMA lands)
    emb_v = emb.rearrange("(p w) d -> p w d", p=P, w=RPP)
    att_v = att.rearrange("(p w) d -> p w d", p=P, w=RPP)
    sizes = [10] + [25] * 9 + [15]
    NA = 4    # rows per chunk reduced via ACT Square+accum instead of DVE
    offs = np.cumsum([0] + sizes).tolist()
    assert offs[-1] == RPP

    Alu = mybir.AluOpType
    Act = mybir.ActivationFunctionType
    Ax = mybir.AxisListType

    with tile.TileContext(nc) as tc:
        with (
            tc.tile_pool(name="io", bufs=5) as io_pool,
            tc.tile_pool(name="tmp", bufs=3) as tmp_pool,
            tc.tile_pool(name="small", bufs=1) as small_pool,
        ):
            eps_t = small_pool.tile([P, 1], dt, tag="eps")
            nc.vector.memset(eps_t[:], EPS_PD)
            rel_t = small_pool.tile([P, RPP], dt, tag="rel")
            junk_t = small_pool.tile([P, D], dt16, tag="junk")

            def folds(s_t, o, n):
                # f16 adds run 2 elem/cycle on DVE; the f32-accum reduce only
                # 1/cycle, so fold D 300->150->75 in f16 first
                f1_t = tmp_pool.tile([P, RPC, 150], dt16, tag="f1")
                nc.vector.tensor_tensor(
                    f1_t[:, 0:n], s_t[:, 0:n, 0:150], s_t[:, 0:n, 150:300],
                    op=Alu.add)
                f2_t = tmp_pool.tile([P, RPC, 75], dt16, tag="f2")
                nc.vector.tensor_tensor(
                    f2_t[:, 0:n], f1_t[:, 0:n, 0:75], f1_t[:, 0:n, 75:150],
                    op=Alu.add)
                nc.vector.tensor_reduce(
                    rel_t[:, o:o + n], f2_t[:, 0:n], axis=Ax.X, op=Alu.add)

            # software pipeline: chunk j's folds are emitted AFTER chunk
            # j+1's subtract. Engine streams execute in order, so without
            # the skew DVE sits idle inside every chunk waiting for ACT's
            # Square (f1 reads it) -- that serial chain gated DMA issue at
            # ~15 us/chunk.
            pend = None
            for j, n in enumerate(sizes):
                o = offs[j]
                e_t = io_pool.tile([P, RPC, D], dt16, tag="e")
                nc.gpsimd.dma_start(e_t[:, 0:n], emb_v[:, o:o + n])
                a_t = io_pool.tile([P, RPC, D], dt16, tag="a")
                nc.gpsimd.dma_start(a_t[:, 0:n], att_v[:, o:o + n])
                # in-place diff then Square back over e: no extra tiles, so
                # the io pool runs deep and DMA issue never waits on compute
                nc.vector.tensor_sub(a_t[:, 0:n], e_t[:, 0:n], a_t[:, 0:n])
                # hybrid row-sum: DVE folds and ACT are both near the DMA
                # rate, so the last NA rows of each chunk go through ACT
                # Square+accum_out ops (ACT had slack) instead of DVE folds
                nf = n - NA
                nc.scalar.activation(e_t[:, 0:nf], a_t[:, 0:nf], Act.Square,
                                     bias=eps_t[:], scale=1.0)
                for w in range(nf, n):
                    nc.scalar.activation(
                        junk_t[:], a_t[:, w, :], Act.Square,
                        bias=eps_t[:], scale=1.0,
                        accum_out=rel_t[:, o + w:o + w + 1])
                if pend is not None:
                    folds(*pend)
                pend = (e_t, o, nf)
            folds(*pend)

            nc.sync.dma_start(out[:], rel_t[:])
    nc.compile()
    return nc


def _get_nc():
    if "nc" not in _STATE:
        _STATE["nc"] = _build()
    return _STATE["nc"]


def _run_device(attributes, embeddings, labels_np, trace=False):
    from concourse.bass_utils import run_bass_kernel_spmd
    nc = _get_nc()
    attributes = np.ascontiguousarray(attributes.astype(np.float16, copy=False))
    embeddings = np.ascontiguousarray(embeddings.astype(np.float16, copy=False))
    in_maps = []
    for m in range(M):
        sl = slice(m * ROWS, (m + 1) * ROWS)
        in_maps.append({
            "emb": embeddings[sl],
            "att": attributes[sl],
        })
    return run_bass_kernel_spmd(nc, in_maps, list(range(M)), trace=trace)


def _combine(results, labels_np):
    """Assemble [B, C] relations from per-core row-sums; finish on host."""
    rel_sq = np.concatenate(
        [np.asarray(r["out"], dtype=np.float64).reshape(ROWS) for r in results]
    ).reshape(B, C)
    relations = np.sqrt(np.maximum(rel_sq, 0.0))
    mask_pos = np.zeros((B, C), dtype=np.float64)
    mask_pos[np.arange(B), labels_np.astype(np.int64)] = 1.0
    hp = (relations * mask_pos).max(axis=0)
    mx = relations.max(axis=0)
    hn = (relations + mx[None, :] * mask_pos).min(axis=0)
    triplet = np.maximum(hp - hn + MARGIN, 0.0)
    num_hard = np.sum(triplet > 1e-16)
    loss = np.sum(triplet) / (num_hard + 1e-16)
    return np.float32(loss)


def kernel(attributes, embeddings, labels):
    attributes = np.asarray(attributes)
    embeddings = np.asarray(embeddings)
    labels_np = np.asarray(labels)
    res = _run_device(attributes, embeddings, labels_np)
    return _combine(res.results, labels_np)


# revision 34
# speedup vs baseline: 1.0076x; 1.0076x over previous
"""HardTripletLoss Trainium2 kernel.

Reference computation (B=256, C=1000, D=300):
  relations[b,c] = ||emb[b*C+c] - att[b*C+c] + 1e-6||_2          [B, C]
  hardest_positive[c] = max_b relations[b,c] * onehot(labels)[b,c]
  mx[c]              = max_b relations[b,c]
  hardest_negative[c] = min_b (relations[b,c] + mx[c]*onehot[b,c])
  loss = sum(relu(hp - hn + 1)) / (count(relu(...) > 1e-16) + 1e-16)

Sharding: data-parallel over B across 8 cores; core m owns the contiguous
32000-row slice [m*32000, (m+1)*32000) of the (B*C, D) tensors. The device
does the heavy part only: per-row squared distances rel_sq[row] =
sum_d (emb-att+eps)^2, shipped back as [128, 250] f32 (128 KB/core,
+0.3% traffic). The host reshapes to [B, C], takes sqrt, and runs the
reference's tiny [B,C] max/min/mask logic in numpy (~ms).

Performance design (HW exec ~= DMA roofline; measured cap ~200 GB/s/core):
- fp16 inputs (host pre-cast): halves device bytes to 38.4 MB/core.
  Final loss err ~3e-5 vs 2e-2 tolerance.
- All bulk loads on SWDGE (gpsimd): packets round-robin over all 16 DMA
  engines (22.5 GB/s each). HWDGE queues are pinned to engines 64-68
  only and bottleneck at ~110 GB/s.
- Rows-contiguous layout: partition p holds rows [p*250, (p+1)*250), so
  a 25-row chunk DMA is [128, 15 KB] -- 128 descriptors, 15 KB packets.
  Fewer descriptors per DMA matters because SWDGE issue lock-steps on a
  small completion-sem ring (~8 DMAs in flight): with 250-desc pair DMAs
  the issue stream stalled 10-14 us at a time; 20 chunk DMAs of 1.92 MB
  keep ~15 MB queued ahead of the engines.
- Per chunk: in-place DVE subtract, one ACT Square (bias=eps), two f16
  fold-adds (2 elem/cycle) + one f32 3D row-sum reduce on DVE. All
  compute engines run far below the DMA window.
"""

import numpy as np

B, C, D = 256, 1000, 300
M = 8              # cores
BL = B // M        # 32 local anchors per core
ROWS = BL * C      # 32000 rows per core
P = 128            # partitions; partition p holds rows [p*RPP, (p+1)*RPP)
RPP = ROWS // P    # 250 rows per partition
RPC = 25           # rows per chunk (per partition)
NCH = RPP // RPC   # 10 chunks
EPS_PD = 1e-6
MARGIN = 1.0

_STATE = {}


def _build():
    import concourse.tile as tile
    from concourse import bacc, mybir

    nc = bacc.Bacc("TRN2", target_bir_lowering=False, debug=False,
                   num_devices=M, num_swdge_queues=4)
    dt = mybir.dt.float32
    dt16 = mybir.dt.float16
    emb = nc.dram_tensor("emb", [ROWS, D], dt16, kind="ExternalInput").ap()
    att = nc.dram_tensor("att", [ROWS, D], dt16, kind="ExternalInput").ap()
    out = nc.dram_tensor("out", [P, RPP], dt, kind="ExternalOutput").ap()

    # row = p*250 + w  ->  per-partition row view [p, w, d]; chunks are
    # uneven: small ones first (compute starts sooner after launch) and
    # last (short serial drain chain after the final DMA lands)
    emb_v = emb.rearrange("(p w) d -> p w d", p=P, w=RPP)
    att_v = att.rearrange("(p w) d -> p w d", p=P, w=RPP)
    sizes = [25] * 9 + [15, 10]
    NA = 4    # rows per chunk reduced via ACT Square+accum instead of DVE
    offs = np.cumsum([0] + sizes).tolist()
    assert offs[-1] == RPP

    Alu = mybir.AluOpType
    Act = mybir.ActivationFunctionType
    Ax = mybir.AxisListType

    with tile.TileContext(nc) as tc:
        with (
            tc.tile_pool(name="io", bufs=5) as io_pool,
            tc.tile_pool(name="tmp", bufs=3) as tmp_pool,
            tc.tile_pool(name="small", bufs=1) as small_pool,
        ):
            eps_t = small_pool.tile([P, 1], dt, tag="eps")
            nc.vector.memset(eps_t[:], EPS_PD)
            rel_t = small_pool.tile([P, RPP], dt, tag="rel")
            junk_t = small_pool.tile([P, D], dt16, tag="junk")

            def folds(s_t, o, n):
                # f16 adds run 2 elem/cycle on DVE; the f32-accum reduce only
                # 1/cycle, so fold D 300->150->75 in f16 first
                f1_t = tmp_pool.tile([P, RPC, 150], dt16, tag="f1")
                nc.vector.tensor_tensor(
                    f1_t[:, 0:n], s_t[:, 0:n, 0:150], s_t[:, 0:n, 150:300],
                    op=Alu.add)
                f2_t = tmp_pool.tile([P, RPC, 75], dt16, tag="f2")
                nc.vector.tensor_tensor(
                    f2_t[:, 0:n], f1_t[:, 0:n, 0:75], f1_t[:, 0:n, 75:150],
                    op=Alu.add)
                nc.vector.tensor_reduce(
                    rel_t[:, o:o + n], f2_t[:, 0:n], axis=Ax.X, op=Alu.add)

            # software pipeline: chunk j's folds are emitted AFTER chunk
            # j+1's subtract. Engine streams execute in order, so without
            # the skew DVE sits idle inside every chunk waiting for ACT's
            # Square (f1 reads it) -- that serial chain gated DMA issue at
            # ~15 us/chunk.
            pend = None
            for j, n in enumerate(sizes):
                o = offs[j]
                e_t = io_pool.tile([P, RPC, D], dt16, tag="e")
                nc.gpsimd.dma_start(e_t[:, 0:n], emb_v[:, o:o + n])
                a_t = io_pool.tile([P, RPC, D], dt16, tag="a")
                nc.gpsimd.dma_start(a_t[:, 0:n], att_v[:, o:o + n])
                # in-place diff then Square back over e: no extra tiles, so
                # the io pool runs deep and DMA issue never waits on compute
                nc.vector.tensor_sub(a_t[:, 0:n], e_t[:, 0:n], a_t[:, 0:n])
                # hybrid row-sum: DVE folds and ACT are both near the DMA
                # rate, so the last NA rows of each chunk go through ACT
                # Square+accum_out ops (ACT had slack) instead of DVE folds
                nf = n - NA
                nc.scalar.activation(e_t[:, 0:nf], a_t[:, 0:nf], Act.Square,
                                     bias=eps_t[:], scale=1.0)
                for w in range(nf, n):
                    nc.scalar.activation(
                        junk_t[:], a_t[:, w, :], Act.Square,
                        bias=eps_t[:], scale=1.0,
                        accum_out=rel_t[:, o + w:o + w + 1])
                if pend is not None:
                    folds(*pend)
                pend = (e_t, o, nf)
            folds(*pend)

            nc.sync.dma_start(out[:], rel_t[:])
    nc.compile()
    return nc


def _get_nc():
    if "nc" not in _STATE:
        _STATE["nc"] = _build()
    return _STATE["nc"]


def _run_device(attributes, embeddings, labels_np, trace=False):
    from concourse.bass_utils import run_bass_kernel_spmd
    nc = _get_nc()
    attributes = np.ascontiguousarray(attributes.astype(np.float16, copy=False))
    embeddings = np.ascontiguousarray(embeddings.astype(np.float16, copy=False))
    in_maps = []
    for m in range(M):
        sl = slice(m * ROWS, (m + 1) * ROWS)
        in_maps.append({
            "emb": embeddings[sl],
            "att": attributes[sl],
        })
    return run_bass_kernel_spmd(nc, in_maps, list(range(M)), trace=trace)


def _combine(results, labels_np):
    """Assemble [B, C] relations from per-core row-sums; finish on host."""
    rel_sq = np.concatenate(
        [np.asarray(r["out"], dtype=np.float64).reshape(ROWS) for r in results]
    ).reshape(B, C)
    relations = np.sqrt(np.maximum(rel_sq, 0.0))
    mask_pos = np.zeros((B, C), dtype=np.float64)
    mask_pos[np.arange(B), labels_np.astype(np.int64)] = 1.0
    hp = (relations * mask_pos).max(axis=0)
    mx = relations.max(axis=0)
    hn = (relations + mx[None, :] * mask_pos).min(axis=0)
    triplet = np.maximum(hp - hn + MARGIN, 0.0)
    num_hard = np.sum(triplet > 1e-16)
    loss = np.sum(triplet) / (num_hard + 1e-16)
    return np.float32(loss)


def kernel(attributes, embeddings, labels):
    attributes = np.asarray(attributes)
    embeddings = np.asarray(embeddings)
    labels_np = np.asarray(labels)
    res = _run_device(attributes, embeddings, labels_np)
    return _combine(res.results, labels_np)


# revision 39
# speedup vs baseline: 1.1092x; 1.1009x over previous
"""HardTripletLoss Trainium2 kernel.

Reference computation (B=256, C=1000, D=300):
  relations[b,c] = ||emb[b*C+c] - att[b*C+c] + 1e-6||_2          [B, C]
  hardest_positive[c] = max_b relations[b,c] * onehot(labels)[b,c]
  mx[c]              = max_b relations[b,c]
  hardest_negative[c] = min_b (relations[b,c] + mx[c]*onehot[b,c])
  loss = sum(relu(hp - hn + 1)) / (count(relu(...) > 1e-16) + 1e-16)

Sharding: data-parallel over B across 8 cores; core m owns the contiguous
32000-row slice [m*32000, (m+1)*32000) of the (B*C, D) tensors. The device
does the heavy part only: per-row squared distances rel_sq[row] =
sum_d (emb-att+eps)^2, shipped back as [128, 250] f32 (128 KB/core,
+0.3% traffic). The host reshapes to [B, C], takes sqrt, and runs the
reference's tiny [B,C] max/min/mask logic in numpy (~ms).

Performance design (HW exec ~128 us vs 610 us baseline):
- fp16 inputs (host pre-cast): halves device bytes to 38.4 MB/core.
  Final loss err ~3e-5 vs 2e-2 tolerance.
- All bulk loads on SWDGE (gpsimd): packets round-robin over all 16 DMA
  engines. HWDGE queues are pinned to engines 64-68 only and bottleneck
  at ~110 GB/s total.
- Rows-contiguous layout: partition p holds rows [p*250, (p+1)*250), so
  a 25-row chunk DMA is [128, 15 KB] -- 128 descriptors, 15 KB packets.
  Packet size sets the per-engine rate: 4.8 KB packets sustain ~200
  GB/s/core, 15 KB packets ~425 GB/s/core. Few descriptors per DMA also
  matters: SWDGE issue lock-steps on an 8-deep completion-sem ring, and
  250-desc DMAs stalled the issue stream 10-14 us at a time.
- Per chunk: in-place DVE subtract, one ACT Square (bias=eps), two f16
  fold-adds (2 elem/cycle) + one f32 3D row-sum reduce on DVE, emitted
  one chunk behind the subtract (engine streams are in-order; without
  the skew DVE idles waiting for ACT inside every chunk). DVE (~95 us
  busy) is the critical path, just over the ~92 us DMA window. Do NOT
  add work to DVE, put writes to rel_t on other engines (cross-engine
  WAW ordering serializes), or split the leading chunks smaller -- all
  measured as ~10 us regressions.
"""

import numpy as np

B, C, D = 256, 1000, 300
M = 8              # cores
BL = B // M        # 32 local anchors per core
ROWS = BL * C      # 32000 rows per core
P = 128            # partitions; partition p holds rows [p*RPP, (p+1)*RPP)
RPP = ROWS // P    # 250 rows per partition
RPC = 25           # rows per chunk (per partition)
NCH = RPP // RPC   # 10 chunks
EPS_PD = 1e-6
MARGIN = 1.0

_STATE = {}


def _build():
    import concourse.tile as tile
    from concourse import bacc, mybir

    nc = bacc.Bacc("TRN2", target_bir_lowering=False, debug=False,
                   num_devices=M, num_swdge_queues=4)
    dt = mybir.dt.float32
    dt16 = mybir.dt.float16
    emb = nc.dram_tensor("emb", [ROWS, D], dt16, kind="ExternalInput").ap()
    att = nc.dram_tensor("att", [ROWS, D], dt16, kind="ExternalInput").ap()
    out = nc.dram_tensor("out", [P, RPP], dt, kind="ExternalOutput").ap()

    # row = p*250 + w  ->  per-partition row view [p, w, d]; the last two
    # chunks are small to shorten the serial drain chain after the final
    # DMA lands (sub -> Square -> folds on the last chunk is the tail)
    emb_v = emb.rearrange("(p w) d -> p w d", p=P, w=RPP)
    att_v = att.rearrange("(p w) d -> p w d", p=P, w=RPP)
    sizes = [25] * 9 + [15, 10]
    offs = np.cumsum([0] + sizes).tolist()
    assert offs[-1] == RPP

    Alu = mybir.AluOpType
    Act = mybir.ActivationFunctionType
    Ax = mybir.AxisListType

    with tile.TileContext(nc) as tc:
        with (
            tc.tile_pool(name="io", bufs=5) as io_pool,
            tc.tile_pool(name="tmp", bufs=3) as tmp_pool,
            tc.tile_pool(name="small", bufs=1) as small_pool,
        ):
            eps_t = small_pool.tile([P, 1], dt, tag="eps")
            nc.vector.memset(eps_t[:], EPS_PD)
            rel_t = small_pool.tile([P, RPP], dt, tag="rel")

            def folds(s_t, o, n):
                # f16 adds run 2 elem/cycle on DVE; the f32-accum reduce only
                # 1/cycle, so fold D 300->150->75 in f16 first
                f1_t = tmp_pool.tile([P, RPC, 150], dt16, tag="f1")
                nc.vector.tensor_tensor(
                    f1_t[:, 0:n], s_t[:, 0:n, 0:150], s_t[:, 0:n, 150:300],
                    op=Alu.add)
                f2_t = tmp_pool.tile([P, RPC, 75], dt16, tag="f2")
                nc.vector.tensor_tensor(
                    f2_t[:, 0:n], f1_t[:, 0:n, 0:75], f1_t[:, 0:n, 75:150],
                    op=Alu.add)
                nc.vector.tensor_reduce(
                    rel_t[:, o:o + n], f2_t[:, 0:n], axis=Ax.X, op=Alu.add)

            # software pipeline: chunk j's folds are emitted AFTER chunk
            # j+1's subtract. Engine streams execute in order, so without
            # the skew DVE sits idle inside every chunk waiting for ACT's
            # Square (f1 reads it) -- that serial chain gated DMA issue at
            # ~15 us/chunk.
            pend = None
            for j, n in enumerate(sizes):
                o = offs[j]
                e_t = io_pool.tile([P, RPC, D], dt16, tag="e")
                nc.gpsimd.dma_start(e_t[:, 0:n], emb_v[:, o:o + n])
                a_t = io_pool.tile([P, RPC, D], dt16, tag="a")
                nc.gpsimd.dma_start(a_t[:, 0:n], att_v[:, o:o + n])
                # in-place diff then Square back over e: no extra tiles, so
                # the io pool runs deep and DMA issue never waits on compute
                nc.vector.tensor_sub(a_t[:, 0:n], e_t[:, 0:n], a_t[:, 0:n])
                nc.scalar.activation(e_t[:, 0:n], a_t[:, 0:n], Act.Square,
                                     bias=eps_t[:], scale=1.0)
                if pend is not None:
                    folds(*pend)
                pend = (e_t, o, n)
            folds(*pend)

            nc.sync.dma_start(out[:], rel_t[:])
    nc.compile()
    return nc


def _get_nc():
    if "nc" not in _STATE:
        _STATE["nc"] = _build()
    return _STATE["nc"]


def _run_device(attributes, embeddings, labels_np, trace=False):
    from concourse.bass_utils import run_bass_kernel_spmd
    nc = _get_nc()
    attributes = np.ascontiguousarray(attributes.astype(np.float16, copy=False))
    embeddings = np.ascontiguousarray(embeddings.astype(np.float16, copy=False))
    in_maps = []
    for m in range(M):
        sl = slice(m * ROWS, (m + 1) * ROWS)
        in_maps.append({
            "emb": embeddings[sl],
            "att": attributes[sl],
        })
    return run_bass_kernel_spmd(nc, in_maps, list(range(M)), trace=trace)


def _combine(results, labels_np):
    """Assemble [B, C] relations from per-core row-sums; finish on host."""
    rel_sq = np.concatenate(
        [np.asarray(r["out"], dtype=np.float64).reshape(ROWS) for r in results]
    ).reshape(B, C)
    relations = np.sqrt(np.maximum(rel_sq, 0.0))
    mask_pos = np.zeros((B, C), dtype=np.float64)
    mask_pos[np.arange(B), labels_np.astype(np.int64)] = 1.0
    hp = (relations * mask_pos).max(axis=0)
    mx = relations.max(axis=0)
    hn = (relations + mx[None, :] * mask_pos).min(axis=0)
    triplet = np.maximum(hp - hn + MARGIN, 0.0)
    num_hard = np.sum(triplet > 1e-16)
    loss = np.sum(triplet) / (num_hard + 1e-16)
    return np.float32(loss)


def kernel(attributes, embeddings, labels):
    attributes = np.asarray(attributes)
    embeddings = np.asarray(embeddings)
    labels_np = np.asarray(labels)
    res = _run_device(attributes, embeddings, labels_np)
    return _combine(res.results, labels_np)


# revision 40
# speedup vs baseline: 1.1109x; 1.0015x over previous
"""HardTripletLoss Trainium2 kernel.

Reference computation (B=256, C=1000, D=300):
  relations[b,c] = ||emb[b*C+c] - att[b*C+c] + 1e-6||_2          [B, C]
  hardest_positive[c] = max_b relations[b,c] * onehot(labels)[b,c]
  mx[c]              = max_b relations[b,c]
  hardest_negative[c] = min_b (relations[b,c] + mx[c]*onehot[b,c])
  loss = sum(relu(hp - hn + 1)) / (count(relu(...) > 1e-16) + 1e-16)

Sharding: data-parallel over B across 8 cores; core m owns the contiguous
32000-row slice [m*32000, (m+1)*32000) of the (B*C, D) tensors. The device
does the heavy part only: per-row squared distances rel_sq[row] =
sum_d (emb-att+eps)^2, shipped back as [128, 250] f32 (128 KB/core,
+0.3% traffic). The host reshapes to [B, C], takes sqrt, and runs the
reference's tiny [B,C] max/min/mask logic in numpy (~ms).

Performance design (HW exec ~128 us vs 610 us baseline):
- fp16 inputs (host pre-cast): halves device bytes to 38.4 MB/core.
  Final loss err ~3e-5 vs 2e-2 tolerance.
- All bulk loads on SWDGE (gpsimd): packets round-robin over all 16 DMA
  engines. HWDGE queues are pinned to engines 64-68 only and bottleneck
  at ~110 GB/s total.
- Rows-contiguous layout: partition p holds rows [p*250, (p+1)*250), so
  a 25-row chunk DMA is [128, 15 KB] -- 128 descriptors, 15 KB packets.
  Packet size sets the per-engine rate: 4.8 KB packets sustain ~200
  GB/s/core, 15 KB packets ~425 GB/s/core. Few descriptors per DMA also
  matters: SWDGE issue lock-steps on an 8-deep completion-sem ring, and
  250-desc DMAs stalled the issue stream 10-14 us at a time.
- Per chunk: in-place DVE subtract, one ACT Square (bias=eps), two f16
  fold-adds (2 elem/cycle) + one f32 3D row-sum reduce on DVE, emitted
  one chunk behind the subtract (engine streams are in-order; without
  the skew DVE idles waiting for ACT inside every chunk). DVE (~95 us
  busy) is the critical path, just over the ~92 us DMA window. Do NOT
  add work to DVE, put writes to rel_t on other engines (cross-engine
  WAW ordering serializes), or split the leading chunks smaller -- all
  measured as ~10 us regressions.
"""

import numpy as np

B, C, D = 256, 1000, 300
M = 8              # cores
BL = B // M        # 32 local anchors per core
ROWS = BL * C      # 32000 rows per core
P = 128            # partitions; partition p holds rows [p*RPP, (p+1)*RPP)
RPP = ROWS // P    # 250 rows per partition
RPC = 25           # rows per chunk (per partition)
NCH = RPP // RPC   # 10 chunks
EPS_PD = 1e-6
MARGIN = 1.0

_STATE = {}


def _build():
    import concourse.tile as tile
    from concourse import bacc, mybir

    nc = bacc.Bacc("TRN2", target_bir_lowering=False, debug=False,
                   num_devices=M, num_swdge_queues=4)
    dt = mybir.dt.float32
    dt16 = mybir.dt.float16
    emb = nc.dram_tensor("emb", [ROWS, D], dt16, kind="ExternalInput").ap()
    att = nc.dram_tensor("att", [ROWS, D], dt16, kind="ExternalInput").ap()
    out = nc.dram_tensor("out", [P, RPP], dt, kind="ExternalOutput").ap()

    # row = p*250 + w  ->  per-partition row view [p, w, d]; the last two
    # chunks are small to shorten the serial drain chain after the final
    # DMA lands (sub -> Square -> folds on the last chunk is the tail)
    emb_v = emb.rearrange("(p w) d -> p w d", p=P, w=RPP)
    att_v = att.rearrange("(p w) d -> p w d", p=P, w=RPP)
    sizes = [25] * 9 + [15, 10]
    offs = np.cumsum([0] + sizes).tolist()
    assert offs[-1] == RPP

    Alu = mybir.AluOpType
    Act = mybir.ActivationFunctionType
    Ax = mybir.AxisListType

    with tile.TileContext(nc) as tc:
        with (
            tc.tile_pool(name="io", bufs=5) as io_pool,
            tc.tile_pool(name="tmp", bufs=4) as tmp_pool,
            tc.tile_pool(name="small", bufs=1) as small_pool,
        ):
            eps_t = small_pool.tile([P, 1], dt, tag="eps")
            nc.vector.memset(eps_t[:], EPS_PD)
            rel_t = small_pool.tile([P, RPP], dt, tag="rel")

            def folds(s_t, o, n):
                # f16 adds run 2 elem/cycle on DVE; the f32-accum reduce only
                # 1/cycle, so fold D 300->150->75 in f16 first
                f1_t = tmp_pool.tile([P, RPC, 150], dt16, tag="f1")
                nc.vector.tensor_tensor(
                    f1_t[:, 0:n], s_t[:, 0:n, 0:150], s_t[:, 0:n, 150:300],
                    op=Alu.add)
                f2_t = tmp_pool.tile([P, RPC, 75], dt16, tag="f2")
                nc.vector.tensor_tensor(
                    f2_t[:, 0:n], f1_t[:, 0:n, 0:75], f1_t[:, 0:n, 75:150],
                    op=Alu.add)
                nc.vector.tensor_reduce(
                    rel_t[:, o:o + n], f2_t[:, 0:n], axis=Ax.X, op=Alu.add)

            # software pipeline: chunk j's folds are emitted AFTER chunk
            # j+1's subtract. Engine streams execute in order, so without
            # the skew DVE sits idle inside every chunk waiting for ACT's
            # Square (f1 reads it) -- that serial chain gated DMA issue at
            # ~15 us/chunk.
            pend = None
            for j, n in enumerate(sizes):
                o = offs[j]
                e_t = io_pool.tile([P, RPC, D], dt16, tag="e")
                nc.gpsimd.dma_start(e_t[:, 0:n], emb_v[:, o:o + n])
                a_t = io_pool.tile([P, RPC, D], dt16, tag="a")
                nc.gpsimd.dma_start(a_t[:, 0:n], att_v[:, o:o + n])
                # in-place diff then Square back over e: no extra tiles, so
                # the io pool runs deep and DMA issue never waits on compute
                nc.vector.tensor_sub(a_t[:, 0:n], e_t[:, 0:n], a_t[:, 0:n])
                nc.scalar.activation(e_t[:, 0:n], a_t[:, 0:n], Act.Square,
                                     bias=eps_t[:], scale=1.0)
                if pend is not None:
                    folds(*pend)
                pend = (e_t, o, n)
            folds(*pend)

            nc.sync.dma_start(out[:], rel_t[:])
    nc.compile()
    return nc


def _get_nc():
    if "nc" not in _STATE:
        _STATE["nc"] = _build()
    return _STATE["nc"]


def _run_device(attributes, embeddings, labels_np, trace=False):
    from concourse.bass_utils import run_bass_kernel_spmd
    nc = _get_nc()
    attributes = np.ascontiguousarray(attributes.astype(np.float16, copy=False))
    embeddings = np.ascontiguousarray(embeddings.astype(np.float16, copy=False))
    in_maps = []
    for m in range(M):
        sl = slice(m * ROWS, (m + 1) * ROWS)
        in_maps.append({
            "emb": embeddings[sl],
            "att": attributes[sl],
        })
    return run_bass_kernel_spmd(nc, in_maps, list(range(M)), trace=trace)


def _combine(results, labels_np):
    """Assemble [B, C] relations from per-core row-sums; finish on host."""
    rel_sq = np.concatenate(
        [np.asarray(r["out"], dtype=np.float64).reshape(ROWS) for r in results]
    ).reshape(B, C)
    relations = np.sqrt(np.maximum(rel_sq, 0.0))
    mask_pos = np.zeros((B, C), dtype=np.float64)
    mask_pos[np.arange(B), labels_np.astype(np.int64)] = 1.0
    hp = (relations * mask_pos).max(axis=0)
    mx = relations.max(axis=0)
    hn = (relations + mx[None, :] * mask_pos).min(axis=0)
    triplet = np.maximum(hp - hn + MARGIN, 0.0)
    num_hard = np.sum(triplet > 1e-16)
    loss = np.sum(triplet) / (num_hard + 1e-16)
    return np.float32(loss)


def kernel(attributes, embeddings, labels):
    attributes = np.asarray(attributes)
    embeddings = np.asarray(embeddings)
    labels_np = np.asarray(labels)
    res = _run_device(attributes, embeddings, labels_np)
    return _combine(res.results, labels_np)
